# revision 22
# baseline (speedup 1.0000x reference)
"""Depthwise causal conv1d kernel for Trainium2 (8 NeuronCores, SPMD).

Problem: x [B=8, T=4096, C=512] f32, weight [C=512, K=4] f32.
out[b, t, c] = sum_k weight[c, k] * x[b, t - 3 + k, c]   (causal, zero-pad)

Strategy:
  - Data-parallel over batch: core b handles x[b].
  - Host-side layout: each core's input is channels-first x[b].T padded with
    K-1 = 3 leading zeros along time -> [C=512, T+3=4099] so the device
    kernel sees contiguous time on the free axis and channels on partitions.
  - Device: an 8 KB weight-column table lands first; GpSimd expands it into
    16 diag(weight[:, k]) [128x128] stationary matrices (f32), one DVE copy
    re-tags them float32r. For each 128-channel chunk the 4-tap conv is 4
    accumulating TensorE matmuls (stationary diag, moving = shifted x
    views). float32r operands give single-pass full-rate PE (fp32 proper is
    4 cycles/row); the ~2^-12 operand rounding is well inside tolerance for
    a 4-tap conv. PSUM results are copied to SBUF (VectorE even chunks,
    ScalarE odd) and stored with 2 MiB DMAs. HBM-bound: ~16.8 MB of
    traffic per core at ~360+ GB/s.
"""

import numpy as np

B, T, C, K = 8, 4096, 512, 4
P = 128  # partitions
NCHUNK = C // P  # 4 channel chunks
TJ = 512  # time-tile (free dim) per matmul; one PSUM bank
NJ = T // TJ  # 8 time tiles per chunk
TP = T + K - 1  # padded time = 4099
NW = NCHUNK * K  # 16 (chunk, tap) pairs

_compiled = None


def _build():
    import concourse.bacc as bacc
    import concourse.bass as bass
    import concourse.mybir as mybir
    from concourse.tile import TileContext

    f32 = mybir.dt.float32
    f32r = mybir.dt.float32r
    nc = bacc.Bacc()

    wt_d = nc.declare_dram_parameter("wt", [P, NW], f32, isOutput=False)
    xw_d = nc.declare_dram_parameter("xw", [P, NCHUNK * TP], f32r, isOutput=False)
    out_d = nc.declare_dram_parameter("out", [C, T], f32, isOutput=True)

    with TileContext(nc) as tc:
        with (
            tc.tile_pool(name="xpool", bufs=1) as xpool,
            tc.tile_pool(name="wpool", bufs=1) as wpool,
            tc.tile_pool(name="opool", bufs=4) as opool,
            tc.tile_pool(name="ppool", bufs=8, space="PSUM") as ppool,
        ):
            # weight table first: tiny DMA, expansion overlaps the x loads
            wcol = wpool.tile([P, NW], f32, tag="wcol")
            nc.sync.dma_start(out=wcol, in_=wt_d[:, :])
            xts = []
            for c in range(NCHUNK):
                xt = xpool.tile([P, TP], f32r, name=f"xt{c}", tag=f"xt{c}")
                nc.sync.dma_start(out=xt, in_=xw_d[:, c * TP : (c + 1) * TP])
                xts.append(xt)

            # expand wcol into 16 diag matrices (f32 on GpSimd), then one
            # DVE copy to give them the float32r tag the PE path needs
            wtf = wpool.tile([P, NW * P], f32, tag="wtf")
            for idx in range(NW):
                wsrc = bass.AP(wcol.tensor, wcol.offset + idx, [[NW, P], [0, P]])
                nc.gpsimd.affine_select(
                    out=wtf[:, idx * P : (idx + 1) * P],
                    in_=wsrc,
                    compare_op=mybir.AluOpType.is_equal,
                    fill=0.0,
                    base=0,
                    # iota[p, i] = p - i; == 0 on the diagonal
                    pattern=[[-1, P]],
                    channel_multiplier=1,
                )
            wtile = wpool.tile([P, NW * P], f32r, tag="wtile")
            nc.vector.tensor_copy(wtile, wtf)

            for chunk in range(NCHUNK):
                xv = xts[chunk]
                ot = opool.tile([P, T], f32, tag="ot")
                for j in range(NJ):
                    pt = ppool.tile([P, TJ], f32, name="pt", tag="pt")
                    for k in range(K):
                        woff = (chunk * K + k) * P
                        nc.tensor.matmul(
                            pt,
                            wtile[:, woff : woff + P],
                            xv[:, j * TJ + k : j * TJ + k + TJ],
                            start=(k == 0),
                            stop=(k == K - 1),
                        )
                    dst = ot[:, j * TJ : (j + 1) * TJ]
                    if chunk % 2 == 0:
                        nc.vector.tensor_copy(dst, pt)
                    else:
                        nc.scalar.copy(dst, pt)
                nc.sync.dma_start(
                    out=out_d[chunk * P : (chunk + 1) * P, :], in_=ot
                )

    nc.compile()
    return nc


def _prep_inputs(x: np.ndarray, weight: np.ndarray):
    # wcol[p, chunk*K + k] = weight[chunk*P + p, k]
    wcol = np.ascontiguousarray(
        weight.reshape(NCHUNK, P, K).transpose(1, 0, 2).reshape(P, NW)
    )
    xs = []
    for b in range(B):
        xp = np.zeros((C, TP), dtype=np.float32)
        xp[:, K - 1 :] = x[b].T  # [512, 4099], 3 leading zeros
        xw = np.ascontiguousarray(
            xp.reshape(NCHUNK, P, TP).transpose(1, 0, 2).reshape(P, NCHUNK * TP)
        )
        xs.append(xw)
    return xs, wcol


def kernel(x: np.ndarray, weight: np.ndarray) -> np.ndarray:
    global _compiled
    from concourse import bass_utils

    x = np.ascontiguousarray(x, dtype=np.float32)
    weight = np.ascontiguousarray(weight, dtype=np.float32)

    if _compiled is None:
        _compiled = _build()
    nc = _compiled

    xs, wcol = _prep_inputs(x, weight)
    in_maps = [{"xw": xs[b], "wt": wcol} for b in range(B)]
    res = bass_utils.run_bass_kernel_spmd(nc, in_maps, core_ids=list(range(B)))

    out = np.empty((B, T, C), dtype=np.float32)
    for b in range(B):
        out[b] = np.asarray(res.results[b]["out"]).T
    return out
